# revision 10
# baseline (speedup 1.0000x reference)
"""Trainium2 Bass kernel for nn_ClusteringLayer (greedy per-cacheline clustering).

Contract: kernel(x) takes the FULL input (64,256,56,56) fp32 and returns the
FULL output, sharding the 802816 cachelines across 8 NeuronCores internally.

Algorithm (per 64-element cacheline, vectorized across 128 partitions x G
lines/partition): single ascending pass over positions s=0..62. A position's
state is encoded in the value itself:
  clean value x      -> not yet matched (and, once s is reached, a base)
  tagged value b*2^60 -> matched to base value b (exact exponent shift)

The whole per-step suffix update runs as ONE custom DVE instruction
(CLUSTER_STEP_ANT, registered below):
  d  = XO[i] - XO[s]
  q  = d*d                       (fp32; q < C_SQ  <=>  |d| < 0.1f exactly,
                                  C_SQ = fp32(0.1)^2 sits in the 2-ulp gap)
  m  = (q < C_SQ) & (q > 0)
  XO[i] <- XO[s]*2^60 where m else XO[i]
No poisoning pass is needed: a tagged source column XO[s] is huge, so q is
huge for clean XO[i]; two elements tagged with the SAME base give d == 0,
excluded by the q > 0 term. Two elements tagged with different bases differ
by >= 0.1*2^60. The only divergence from the reference is d*d underflow
(|d| < ~2^-63), which changes values by < 1e-19 — bitwise-validated equal on
the full fixed dataset in numpy emulation.

Final pass untags in one more custom op: XO = XO*2^-60 where XO^2 >= 1e12.
Input stats (fixed seed): min|x|=7.5e-8 -> min tag 8.6e10 >> sqrt(1e12);
max|x|=5.42 -> tag^2 = 3.9e37 < fp32 max.
"""

from contextlib import ExitStack

import numpy as np

import concourse.bass as bass
import concourse.tile as tile
from concourse import mybir
from concourse._compat import with_exitstack
from concourse.bass_utils import run_bass_kernel_spmd
from concourse.dve_ops import (
    OPS,
    CUSTOM_DVE_SPECS,
    DveOp,
    _SUB_OPCODE_FOR_NAME,
)
from concourse.dve_spec import C0, C1, Spec, Src0, Src1, Zero, select, sq

N_CORES = 8
CL = 64
FULL_SHAPE = (64, 256, 56, 56)
N_LINES = int(np.prod(FULL_SHAPE)) // CL  # 802816
LINES_PER_CORE = N_LINES // N_CORES  # 100352
TAG = float(2.0**60)
UNTAG = float(2.0**-60)
C_SQ = float(np.float32(0.1) * np.float32(0.1))  # 0.010000001; pass iff q < C_SQ
SQ_THR = 1.0e12  # XO^2 >= this <=> tagged (clean^2 <= ~30, tagged^2 >= 7e21)
F32 = mybir.dt.float32


def _np_cluster_step(in0, in1, s0, s1, imm2):
    d = in0.astype(np.float32) - in1
    q = d * d
    m = (q < np.float32(s0)) & (q > 0)
    return np.where(m, (in1 * np.float32(TAG)).astype(np.float32), in0).astype(
        np.float32
    )


def _np_cluster_untag(in0, in1, s0, s1, imm2):
    q = in0.astype(np.float32) * in0
    return np.where(q < np.float32(s0), in0, (in0 * np.float32(s1))).astype(np.float32)


_d = Src0 - Src1
_q = sq(_d)
CLUSTER_STEP_ANT = DveOp(
    "CLUSTER_STEP_ANT",
    Spec(
        body=select((_q < C0) & (_q > Zero), Src1 * C1, Src0),
        reference=_np_cluster_step,
    ),
    subdim=False,
    uops_sha={"v3": "7b109a958a4303df", "v4": "186ea62efc972dc3"},
)

_q0 = sq(Src0)
CLUSTER_UNTAG_ANT = DveOp(
    "CLUSTER_UNTAG_ANT",
    Spec(
        body=select(_q0 < C0, Src0, Src0 * C1),
        reference=_np_cluster_untag,
    ),
    subdim=False,
    uops_sha={"v3": None, "v4": None},  # filled in at import below
)


def _register_ops():
    from concourse.dve_spec import _has_src1, lower
    from concourse.dve_uop import DveOpSpec

    for op in (CLUSTER_STEP_ANT, CLUSTER_UNTAG_ANT):
        if op.name in _SUB_OPCODE_FOR_NAME:
            continue
        row = max(_SUB_OPCODE_FOR_NAME.values()) + 1
        assert row < 0x20, "custom-DVE row field overflow"
        _SUB_OPCODE_FOR_NAME[op.name] = row
        OPS.append(op)
        CUSTOM_DVE_SPECS[op.name] = op.spec
        # pin the real uop shas (DveOp.compile asserts against these), computed
        # exactly the way DveOp.compile builds its DveOpSpec
        for ver in ("v3", "v4"):
            got = DveOpSpec(
                name=op.name,
                opcode=row,
                uops=lower(op.spec, ver=ver),
                rd1_en=_has_src1(op.spec),
            ).sha(ver)
            op.uops_sha[ver] = got


_register_ops()


def _bcast(col_ap: bass.AP, span: int) -> bass.AP:
    """View a (P, G) column AP as (P, G, span) with stride-0 innermost dim."""
    ap_rows = [list(r) for r in col_ap.ap]
    return bass.AP(
        tensor=col_ap.tensor,
        offset=col_ap.offset,
        ap=ap_rows + [[0, span]],
    )


@with_exitstack
def _cluster_kernel(
    ctx: ExitStack,
    tc: tile.TileContext,
    out_ap: bass.AP,
    in_ap: bass.AP,
    G: int,
    n_tiles: int,
    bufs: int = 2,
    untag_on_gpsimd: bool = False,
):
    nc = tc.nc
    lines_per_tile = 128 * G
    Alu = mybir.AluOpType

    xpool = ctx.enter_context(tc.tile_pool(name="xpool", bufs=bufs))
    if untag_on_gpsimd:
        spool = ctx.enter_context(tc.tile_pool(name="spool", bufs=bufs))
        cpool = ctx.enter_context(tc.tile_pool(name="cpool", bufs=1))
        THRC = cpool.tile([128, 1], F32, tag="thr")
        UNTC = cpool.tile([128, 1], F32, tag="unt")
        nc.vector.memset(THRC[:], SQ_THR)
        nc.vector.memset(UNTC[:], UNTAG)

    for t in range(n_tiles):
        r0 = t * lines_per_tile
        src = in_ap[r0 : r0 + lines_per_tile, :].rearrange("(p g) c -> p g c", p=128)
        XO = xpool.tile([128, G, CL], F32, tag="xo")
        nc.sync.dma_start(out=XO[:, :, :], in_=src)
        for s in range(CL - 1):
            espan = CL - 1 - s
            nc.vector._custom_dve(
                CLUSTER_STEP_ANT,
                out=XO[:, :, s + 1 : CL],
                in0=XO[:, :, s + 1 : CL],
                in1=_bcast(XO[:, :, s], espan),
                s0=C_SQ,
                s1=TAG,
            )
        if untag_on_gpsimd:
            # Untag on the otherwise-idle GPSIMD engine so it overlaps the
            # next tile's DVE steps. Exact: clean part passes through as
            # XO*1.0; tagged part is XO*2^-60 (power-of-two, exact).
            Q = spool.tile([128, G, CL], F32, tag="q")
            T1 = spool.tile([128, G, CL], F32, tag="t1")
            g = nc.gpsimd

            def cb(col, span):
                return _bcast(_bcast(col, G), span)

            XOf = XO[:, :, :]
            g.tensor_tensor(out=Q[:], in0=XOf, in1=XOf, op=Alu.mult)  # XO^2
            g.tensor_tensor(out=T1[:], in0=Q[:], in1=cb(THRC[:, 0], CL), op=Alu.is_lt)
            g.tensor_tensor(out=T1[:], in0=T1[:], in1=XOf, op=Alu.mult)  # clean part
            g.tensor_tensor(out=Q[:], in0=Q[:], in1=cb(THRC[:, 0], CL), op=Alu.is_ge)
            g.tensor_tensor(out=XO[:, :, :], in0=XOf, in1=cb(UNTC[:, 0], CL), op=Alu.mult)
            g.tensor_tensor(out=Q[:], in0=Q[:], in1=XOf, op=Alu.mult)  # tagged part
            g.tensor_tensor(out=XO[:, :, :], in0=T1[:], in1=Q[:], op=Alu.add)
        else:
            nc.vector._custom_dve(
                CLUSTER_UNTAG_ANT,
                out=XO[:, :, :],
                in0=XO[:, :, :],
                s0=SQ_THR,
                s1=UNTAG,
            )
        dst = out_ap[r0 : r0 + lines_per_tile, :].rearrange("(p g) c -> p g c", p=128)
        nc.sync.dma_start(out=dst, in_=XO[:, :, :])


def _split_multi_waits(nc: bass.Bass, max_waits: int = 1) -> None:
    """walrus CoreV3 codegen rejects instructions with more than one or two
    sync-wait conditions ("Too many sync wait commands"). Split extra waits
    onto single-wait NOPs inserted just before the instruction (same engine,
    same block) — semantically identical for monotonic semaphores."""

    def walk(blocks):
        for bb in blocks:
            yield bb
            sub = getattr(bb, "blocks", None)
            if sub:
                yield from walk(sub)

    for fn in nc.m.functions:
        for bb in walk(fn.blocks):
            out = []
            changed = False
            for inst in bb.instructions:
                si = inst.sync_info
                if si is not None and si.on_wait and len(si.on_wait) > max_waits:
                    waits = list(si.on_wait)
                    head, tail = waits[:-max_waits], waits[-max_waits:]
                    for k, w in enumerate(head):
                        out.append(
                            mybir.InstNoOp(
                                name=f"{inst.name}-w{k}",
                                engine=inst.engine,
                                bass_nofuse=True,
                                sync_info=mybir.SyncInfo(on_wait=[w], on_update=[]),
                            )
                        )
                    inst.sync_info = mybir.SyncInfo(
                        on_wait=tail, on_update=list(si.on_update)
                    )
                    changed = True
                out.append(inst)
            if changed:
                bb.instructions = out


def build_program(
    lines_per_core: int = LINES_PER_CORE, G: int = 98, bufs: int = 2,
    untag_on_gpsimd: bool = False,
) -> bass.Bass:
    assert lines_per_core % (128 * G) == 0
    n_tiles = lines_per_core // (128 * G)
    nc = bass.Bass("TRN2", target_bir_lowering=False, debug=False)
    xin = nc.declare_dram_parameter("xin", [lines_per_core, CL], F32, isOutput=False)
    yout = nc.declare_dram_parameter("yout", [lines_per_core, CL], F32, isOutput=True)
    with tile.TileContext(nc) as tc:
        _cluster_kernel(tc, yout, xin, G, n_tiles, bufs=bufs,
                        untag_on_gpsimd=untag_on_gpsimd)
    # Raw Bass skips the pass that populates .instr bytes for InstISA
    # subclasses (incl. InstCustomDveAnt); without it walrus codegen fails
    # with "ISA wrong length" (see library_overlay.lower_extended_insts).
    mybir.codegen_inst_isa_subclasses(nc)
    _split_multi_waits(nc)
    return nc


_PROGRAM_CACHE: dict = {}


def _get_program(
    lines_per_core: int, G: int, bufs: int = 2, untag_on_gpsimd: bool = False
) -> bass.Bass:
    key = (lines_per_core, G, bufs, untag_on_gpsimd)
    if key not in _PROGRAM_CACHE:
        _PROGRAM_CACHE[key] = build_program(lines_per_core, G, bufs, untag_on_gpsimd)
    return _PROGRAM_CACHE[key]


def run_sharded(flat_lines: np.ndarray, G: int = 98, trace: bool = False, bufs: int = 4,
                untag_on_gpsimd: bool = False):
    """flat_lines: (n_lines, 64) fp32 with n_lines divisible by N_CORES*128*G.
    Returns (out_lines, BassKernelResults)."""
    n_lines = flat_lines.shape[0]
    lines_per_core = n_lines // N_CORES
    nc = _get_program(lines_per_core, G, bufs, untag_on_gpsimd)
    in_maps = [
        {"xin": np.ascontiguousarray(flat_lines[c * lines_per_core : (c + 1) * lines_per_core])}
        for c in range(N_CORES)
    ]
    res = run_bass_kernel_spmd(nc, in_maps, list(range(N_CORES)), trace=trace)
    out = np.concatenate([res.results[c]["yout"] for c in range(N_CORES)], axis=0)
    return out, res


def kernel(x: np.ndarray) -> np.ndarray:
    x = np.ascontiguousarray(x, dtype=np.float32)
    flat = x.reshape(-1, CL)
    out, _ = run_sharded(flat, G=98, bufs=4, trace=False)
    return out.reshape(FULL_SHAPE).astype(np.float32)


# revision 12
# speedup vs baseline: 1.0003x; 1.0003x over previous
"""Trainium2 Bass kernel for nn_ClusteringLayer (greedy per-cacheline clustering).

Contract: kernel(x) takes the FULL input (64,256,56,56) fp32 and returns the
FULL output, sharding the 802816 cachelines across 8 NeuronCores internally.

Algorithm (per 64-element cacheline, vectorized across 128 partitions x G
lines/partition): single ascending pass over positions s=0..62. A position's
state is encoded in the value itself:
  clean value x      -> not yet matched (and, once s is reached, a base)
  tagged value b*2^60 -> matched to base value b (exact exponent shift)

The whole per-step suffix update runs as ONE custom DVE instruction
(CLUSTER_STEP_ANT, registered below):
  d  = XO[i] - XO[s]
  q  = d*d                       (fp32; q < C_SQ  <=>  |d| < 0.1f exactly,
                                  C_SQ = fp32(0.1)^2 sits in the 2-ulp gap)
  m  = (q < C_SQ) & (q > 0)
  XO[i] <- XO[s]*2^60 where m else XO[i]
No poisoning pass is needed: a tagged source column XO[s] is huge, so q is
huge for clean XO[i]; two elements tagged with the SAME base give d == 0,
excluded by the q > 0 term. Two elements tagged with different bases differ
by >= 0.1*2^60. The only divergence from the reference is d*d underflow
(|d| < ~2^-63), which changes values by < 1e-19 — bitwise-validated equal on
the full fixed dataset in numpy emulation.

Final pass untags in one more custom op: XO = XO*2^-60 where XO^2 >= 1e12.
Input stats (fixed seed): min|x|=7.5e-8 -> min tag 8.6e10 >> sqrt(1e12);
max|x|=5.42 -> tag^2 = 3.9e37 < fp32 max.
"""

from contextlib import ExitStack

import numpy as np

import concourse.bass as bass
import concourse.tile as tile
from concourse import mybir
from concourse._compat import with_exitstack
from concourse.bass_utils import run_bass_kernel_spmd
from concourse.dve_ops import (
    OPS,
    CUSTOM_DVE_SPECS,
    DveOp,
    _SUB_OPCODE_FOR_NAME,
)
from concourse.dve_spec import C0, C1, Spec, Src0, Src1, Zero, select, sq

N_CORES = 8
CL = 64
FULL_SHAPE = (64, 256, 56, 56)
N_LINES = int(np.prod(FULL_SHAPE)) // CL  # 802816
LINES_PER_CORE = N_LINES // N_CORES  # 100352
TAG = float(2.0**60)
UNTAG = float(2.0**-60)
C_SQ = float(np.float32(0.1) * np.float32(0.1))  # 0.010000001; pass iff q < C_SQ
SQ_THR = 1.0e12  # XO^2 >= this <=> tagged (clean^2 <= ~30, tagged^2 >= 7e21)
F32 = mybir.dt.float32


def _np_cluster_step(in0, in1, s0, s1, imm2):
    d = in0.astype(np.float32) - in1
    q = d * d
    m = (q < np.float32(s0)) & (q > 0)
    return np.where(m, (in1 * np.float32(TAG)).astype(np.float32), in0).astype(
        np.float32
    )


def _np_cluster_untag(in0, in1, s0, s1, imm2):
    q = in0.astype(np.float32) * in0
    return np.where(q < np.float32(s0), in0, (in0 * np.float32(s1))).astype(np.float32)


_d = Src0 - Src1
_q = sq(_d)
CLUSTER_STEP_ANT = DveOp(
    "CLUSTER_STEP_ANT",
    Spec(
        body=select((_q < C0) & (_q > Zero), Src1 * C1, Src0),
        reference=_np_cluster_step,
    ),
    subdim=False,
    uops_sha={"v3": "7b109a958a4303df", "v4": "186ea62efc972dc3"},
)

_q0 = sq(Src0)
CLUSTER_UNTAG_ANT = DveOp(
    "CLUSTER_UNTAG_ANT",
    Spec(
        body=select(_q0 < C0, Src0, Src0 * C1),
        reference=_np_cluster_untag,
    ),
    subdim=False,
    uops_sha={"v3": None, "v4": None},  # filled in at import below
)


def _register_ops():
    from concourse.dve_spec import _has_src1, lower
    from concourse.dve_uop import DveOpSpec

    for op in (CLUSTER_STEP_ANT, CLUSTER_UNTAG_ANT):
        if op.name in _SUB_OPCODE_FOR_NAME:
            continue
        row = max(_SUB_OPCODE_FOR_NAME.values()) + 1
        assert row < 0x20, "custom-DVE row field overflow"
        _SUB_OPCODE_FOR_NAME[op.name] = row
        OPS.append(op)
        CUSTOM_DVE_SPECS[op.name] = op.spec
        # pin the real uop shas (DveOp.compile asserts against these), computed
        # exactly the way DveOp.compile builds its DveOpSpec
        for ver in ("v3", "v4"):
            got = DveOpSpec(
                name=op.name,
                opcode=row,
                uops=lower(op.spec, ver=ver),
                rd1_en=_has_src1(op.spec),
            ).sha(ver)
            op.uops_sha[ver] = got


_register_ops()


def _bcast(col_ap: bass.AP, span: int) -> bass.AP:
    """View a (P, G) column AP as (P, G, span) with stride-0 innermost dim."""
    ap_rows = [list(r) for r in col_ap.ap]
    return bass.AP(
        tensor=col_ap.tensor,
        offset=col_ap.offset,
        ap=ap_rows + [[0, span]],
    )


@with_exitstack
def _cluster_kernel(
    ctx: ExitStack,
    tc: tile.TileContext,
    out_ap: bass.AP,
    in_ap: bass.AP,
    G: int,
    n_tiles: int,
    bufs: int = 2,
    untag_on_gpsimd: bool = False,
):
    nc = tc.nc
    lines_per_tile = 128 * G
    Alu = mybir.AluOpType

    xpool = ctx.enter_context(tc.tile_pool(name="xpool", bufs=bufs))
    if untag_on_gpsimd:
        spool = ctx.enter_context(tc.tile_pool(name="spool", bufs=bufs))
        cpool = ctx.enter_context(tc.tile_pool(name="cpool", bufs=1))
        THRC = cpool.tile([128, 1], F32, tag="thr")
        UNTC = cpool.tile([128, 1], F32, tag="unt")
        nc.vector.memset(THRC[:], SQ_THR)
        nc.vector.memset(UNTC[:], UNTAG)

    g_sched = [G] * n_tiles
    r0 = 0
    for t, Gt in enumerate(g_sched):
        lines_per_tile = 128 * Gt
        src = in_ap[r0 : r0 + lines_per_tile, :].rearrange("(p g) c -> p g c", p=128)
        XO = xpool.tile([128, Gt, CL], F32, tag="xo")
        nc.sync.dma_start(out=XO[:, :, :], in_=src)
        for s in range(CL - 1):
            espan = CL - 1 - s
            nc.vector._custom_dve(
                CLUSTER_STEP_ANT,
                out=XO[:, :, s + 1 : CL],
                in0=XO[:, :, s + 1 : CL],
                in1=_bcast(XO[:, :, s], espan),
                s0=C_SQ,
                s1=TAG,
            )
        if untag_on_gpsimd:
            # Untag on the otherwise-idle GPSIMD engine so it overlaps the
            # next tile's DVE steps. Exact: clean part passes through as
            # XO*1.0; tagged part is XO*2^-60 (power-of-two, exact).
            Q = spool.tile([128, Gt, CL], F32, tag="q")
            T1 = spool.tile([128, Gt, CL], F32, tag="t1")
            g = nc.gpsimd

            def cb(col, span):
                return _bcast(_bcast(col, Gt), span)

            XOf = XO[:, :, :]
            g.tensor_tensor(out=Q[:], in0=XOf, in1=XOf, op=Alu.mult)  # XO^2
            g.tensor_tensor(out=T1[:], in0=Q[:], in1=cb(THRC[:, 0], CL), op=Alu.is_lt)
            g.tensor_tensor(out=T1[:], in0=T1[:], in1=XOf, op=Alu.mult)  # clean part
            g.tensor_tensor(out=Q[:], in0=Q[:], in1=cb(THRC[:, 0], CL), op=Alu.is_ge)
            g.tensor_tensor(out=XO[:, :, :], in0=XOf, in1=cb(UNTC[:, 0], CL), op=Alu.mult)
            g.tensor_tensor(out=Q[:], in0=Q[:], in1=XOf, op=Alu.mult)  # tagged part
            g.tensor_tensor(out=XO[:, :, :], in0=T1[:], in1=Q[:], op=Alu.add)
        else:
            nc.vector._custom_dve(
                CLUSTER_UNTAG_ANT,
                out=XO[:, :, :],
                in0=XO[:, :, :],
                s0=SQ_THR,
                s1=UNTAG,
            )
        dst = out_ap[r0 : r0 + lines_per_tile, :].rearrange("(p g) c -> p g c", p=128)
        nc.sync.dma_start(out=dst, in_=XO[:, :, :])
        r0 += lines_per_tile


def _split_multi_waits(nc: bass.Bass, max_waits: int = 1) -> None:
    """walrus CoreV3 codegen rejects instructions with more than one or two
    sync-wait conditions ("Too many sync wait commands"). Split extra waits
    onto single-wait NOPs inserted just before the instruction (same engine,
    same block) — semantically identical for monotonic semaphores."""

    def walk(blocks):
        for bb in blocks:
            yield bb
            sub = getattr(bb, "blocks", None)
            if sub:
                yield from walk(sub)

    for fn in nc.m.functions:
        for bb in walk(fn.blocks):
            out = []
            changed = False
            for inst in bb.instructions:
                si = inst.sync_info
                if si is not None and si.on_wait and len(si.on_wait) > max_waits:
                    waits = list(si.on_wait)
                    head, tail = waits[:-max_waits], waits[-max_waits:]
                    for k, w in enumerate(head):
                        out.append(
                            mybir.InstNoOp(
                                name=f"{inst.name}-w{k}",
                                engine=inst.engine,
                                bass_nofuse=True,
                                sync_info=mybir.SyncInfo(on_wait=[w], on_update=[]),
                            )
                        )
                    inst.sync_info = mybir.SyncInfo(
                        on_wait=tail, on_update=list(si.on_update)
                    )
                    changed = True
                out.append(inst)
            if changed:
                bb.instructions = out


def build_program(
    lines_per_core: int = LINES_PER_CORE, G: int = 98, bufs: int = 2,
    untag_on_gpsimd: bool = False,
) -> bass.Bass:
    assert lines_per_core % (128 * G) == 0
    n_tiles = lines_per_core // (128 * G)
    nc = bass.Bass("TRN2", target_bir_lowering=False, debug=False)
    xin = nc.declare_dram_parameter("xin", [lines_per_core, CL], F32, isOutput=False)
    yout = nc.declare_dram_parameter("yout", [lines_per_core, CL], F32, isOutput=True)
    with tile.TileContext(nc) as tc:
        _cluster_kernel(tc, yout, xin, G, n_tiles, bufs=bufs,
                        untag_on_gpsimd=untag_on_gpsimd)
    # Raw Bass skips the pass that populates .instr bytes for InstISA
    # subclasses (incl. InstCustomDveAnt); without it walrus codegen fails
    # with "ISA wrong length" (see library_overlay.lower_extended_insts).
    mybir.codegen_inst_isa_subclasses(nc)
    _split_multi_waits(nc)
    return nc


_PROGRAM_CACHE: dict = {}


def _get_program(
    lines_per_core: int, G: int, bufs: int = 2, untag_on_gpsimd: bool = False
) -> bass.Bass:
    key = (lines_per_core, G, bufs, untag_on_gpsimd)
    if key not in _PROGRAM_CACHE:
        _PROGRAM_CACHE[key] = build_program(lines_per_core, G, bufs, untag_on_gpsimd)
    return _PROGRAM_CACHE[key]


def run_sharded(flat_lines: np.ndarray, G: int = 98, trace: bool = False, bufs: int = 4,
                untag_on_gpsimd: bool = False):
    """flat_lines: (n_lines, 64) fp32 with n_lines divisible by N_CORES*128*G.
    Returns (out_lines, BassKernelResults)."""
    n_lines = flat_lines.shape[0]
    lines_per_core = n_lines // N_CORES
    nc = _get_program(lines_per_core, G, bufs, untag_on_gpsimd)
    in_maps = [
        {"xin": np.ascontiguousarray(flat_lines[c * lines_per_core : (c + 1) * lines_per_core])}
        for c in range(N_CORES)
    ]
    res = run_bass_kernel_spmd(nc, in_maps, list(range(N_CORES)), trace=trace)
    out = np.concatenate([res.results[c]["yout"] for c in range(N_CORES)], axis=0)
    return out, res


def kernel(x: np.ndarray) -> np.ndarray:
    x = np.ascontiguousarray(x, dtype=np.float32)
    flat = x.reshape(-1, CL)
    out, _ = run_sharded(flat, G=98, bufs=4, trace=False)
    return out.reshape(FULL_SHAPE).astype(np.float32)


# revision 14
# speedup vs baseline: 1.0003x; 1.0001x over previous
"""Trainium2 Bass kernel for nn_ClusteringLayer (greedy per-cacheline clustering).

Contract: kernel(x) takes the FULL input (64,256,56,56) fp32 and returns the
FULL output, sharding the 802816 cachelines across 8 NeuronCores internally.

Algorithm (per 64-element cacheline, vectorized across 128 partitions x G
lines/partition): single ascending pass over positions s=0..62. A position's
state is encoded in the value itself:
  clean value x      -> not yet matched (and, once s is reached, a base)
  tagged value b*2^60 -> matched to base value b (exact exponent shift)

The whole per-step suffix update runs as ONE custom DVE instruction
(CLUSTER_STEP_ANT, registered below):
  d  = XO[i] - XO[s]
  q  = d*d                       (fp32; q < C_SQ  <=>  |d| < 0.1f exactly,
                                  C_SQ = fp32(0.1)^2 sits in the 2-ulp gap)
  m  = (q < C_SQ) & (q > 0)
  XO[i] <- XO[s]*2^60 where m else XO[i]
No poisoning pass is needed: a tagged source column XO[s] is huge, so q is
huge for clean XO[i]; two elements tagged with the SAME base give d == 0,
excluded by the q > 0 term. Two elements tagged with different bases differ
by >= 0.1*2^60. The only divergence from the reference is d*d underflow
(|d| < ~2^-63), which changes values by < 1e-19 — bitwise-validated equal on
the full fixed dataset in numpy emulation.

Final pass untags in one more custom op: XO = XO*2^-60 where XO^2 >= 1e12.
Input stats (fixed seed): min|x|=7.5e-8 -> min tag 8.6e10 >> sqrt(1e12);
max|x|=5.42 -> tag^2 = 3.9e37 < fp32 max.
"""

from contextlib import ExitStack

import numpy as np

import concourse.bass as bass
import concourse.tile as tile
from concourse import mybir
from concourse._compat import with_exitstack
from concourse.bass_utils import run_bass_kernel_spmd
from concourse.dve_ops import (
    OPS,
    CUSTOM_DVE_SPECS,
    DveOp,
    _SUB_OPCODE_FOR_NAME,
)
from concourse.dve_spec import C0, C1, Spec, Src0, Src1, Zero, select, sq

N_CORES = 8
CL = 64
FULL_SHAPE = (64, 256, 56, 56)
N_LINES = int(np.prod(FULL_SHAPE)) // CL  # 802816
LINES_PER_CORE = N_LINES // N_CORES  # 100352
TAG = float(2.0**60)
UNTAG = float(2.0**-60)
C_SQ = float(np.float32(0.1) * np.float32(0.1))  # 0.010000001; pass iff q < C_SQ
SQ_THR = 1.0e12  # XO^2 >= this <=> tagged (clean^2 <= ~30, tagged^2 >= 7e21)
F32 = mybir.dt.float32


def _np_cluster_step(in0, in1, s0, s1, imm2):
    d = in0.astype(np.float32) - in1
    q = d * d
    m = (q < np.float32(s0)) & (q > 0)
    return np.where(m, (in1 * np.float32(TAG)).astype(np.float32), in0).astype(
        np.float32
    )


def _np_cluster_untag(in0, in1, s0, s1, imm2):
    q = in0.astype(np.float32) * in0
    return np.where(q < np.float32(s0), in0, (in0 * np.float32(s1))).astype(np.float32)


_d = Src0 - Src1
_q = sq(_d)
CLUSTER_STEP_ANT = DveOp(
    "CLUSTER_STEP_ANT",
    Spec(
        body=select((_q < C0) & (_q > Zero), Src1 * C1, Src0),
        reference=_np_cluster_step,
    ),
    subdim=False,
    uops_sha={"v3": "7b109a958a4303df", "v4": "186ea62efc972dc3"},
)

_q0 = sq(Src0)
CLUSTER_UNTAG_ANT = DveOp(
    "CLUSTER_UNTAG_ANT",
    Spec(
        body=select(_q0 < C0, Src0, Src0 * C1),
        reference=_np_cluster_untag,
    ),
    subdim=False,
    uops_sha={"v3": None, "v4": None},  # filled in at import below
)


def _register_ops():
    from concourse.dve_spec import _has_src1, lower
    from concourse.dve_uop import DveOpSpec

    for op in (CLUSTER_STEP_ANT, CLUSTER_UNTAG_ANT):
        if op.name in _SUB_OPCODE_FOR_NAME:
            continue
        row = max(_SUB_OPCODE_FOR_NAME.values()) + 1
        assert row < 0x20, "custom-DVE row field overflow"
        _SUB_OPCODE_FOR_NAME[op.name] = row
        OPS.append(op)
        CUSTOM_DVE_SPECS[op.name] = op.spec
        # pin the real uop shas (DveOp.compile asserts against these), computed
        # exactly the way DveOp.compile builds its DveOpSpec
        for ver in ("v3", "v4"):
            got = DveOpSpec(
                name=op.name,
                opcode=row,
                uops=lower(op.spec, ver=ver),
                rd1_en=_has_src1(op.spec),
            ).sha(ver)
            op.uops_sha[ver] = got


_register_ops()


def _bcast(col_ap: bass.AP, span: int) -> bass.AP:
    """View a (P, G) column AP as (P, G, span) with stride-0 innermost dim."""
    ap_rows = [list(r) for r in col_ap.ap]
    return bass.AP(
        tensor=col_ap.tensor,
        offset=col_ap.offset,
        ap=ap_rows + [[0, span]],
    )


@with_exitstack
def _cluster_kernel(
    ctx: ExitStack,
    tc: tile.TileContext,
    out_ap: bass.AP,
    in_ap: bass.AP,
    G: int,
    n_tiles: int,
    bufs: int = 2,
    untag_on_gpsimd: bool = False,
    split_first: bool = False,
):
    nc = tc.nc
    lines_per_tile = 128 * G
    Alu = mybir.AluOpType

    xpool = ctx.enter_context(tc.tile_pool(name="xpool", bufs=bufs))
    if untag_on_gpsimd:
        spool = ctx.enter_context(tc.tile_pool(name="spool", bufs=bufs))
        cpool = ctx.enter_context(tc.tile_pool(name="cpool", bufs=1))
        THRC = cpool.tile([128, 1], F32, tag="thr")
        UNTC = cpool.tile([128, 1], F32, tag="unt")
        nc.vector.memset(THRC[:], SQ_THR)
        nc.vector.memset(UNTC[:], UNTAG)

    if split_first and n_tiles >= 2 and G % 2 == 0:
        g_sched = [G // 2, G // 2] + [G] * (n_tiles - 1)
    else:
        g_sched = [G] * n_tiles
    r0 = 0
    for t, Gt in enumerate(g_sched):
        lines_per_tile = 128 * Gt
        src = in_ap[r0 : r0 + lines_per_tile, :].rearrange("(p g) c -> p g c", p=128)
        XO = xpool.tile([128, Gt, CL], F32, tag="xo")
        nc.sync.dma_start(out=XO[:, :, :], in_=src)
        for s in range(CL - 1):
            espan = CL - 1 - s
            nc.vector._custom_dve(
                CLUSTER_STEP_ANT,
                out=XO[:, :, s + 1 : CL],
                in0=XO[:, :, s + 1 : CL],
                in1=_bcast(XO[:, :, s], espan),
                s0=C_SQ,
                s1=TAG,
            )
        if untag_on_gpsimd:
            # Untag on the otherwise-idle GPSIMD engine so it overlaps the
            # next tile's DVE steps. Exact: clean part passes through as
            # XO*1.0; tagged part is XO*2^-60 (power-of-two, exact).
            Q = spool.tile([128, Gt, CL], F32, tag="q")
            T1 = spool.tile([128, Gt, CL], F32, tag="t1")
            g = nc.gpsimd

            def cb(col, span):
                return _bcast(_bcast(col, Gt), span)

            XOf = XO[:, :, :]
            g.tensor_tensor(out=Q[:], in0=XOf, in1=XOf, op=Alu.mult)  # XO^2
            g.tensor_tensor(out=T1[:], in0=Q[:], in1=cb(THRC[:, 0], CL), op=Alu.is_lt)
            g.tensor_tensor(out=T1[:], in0=T1[:], in1=XOf, op=Alu.mult)  # clean part
            g.tensor_tensor(out=Q[:], in0=Q[:], in1=cb(THRC[:, 0], CL), op=Alu.is_ge)
            g.tensor_tensor(out=XO[:, :, :], in0=XOf, in1=cb(UNTC[:, 0], CL), op=Alu.mult)
            g.tensor_tensor(out=Q[:], in0=Q[:], in1=XOf, op=Alu.mult)  # tagged part
            g.tensor_tensor(out=XO[:, :, :], in0=T1[:], in1=Q[:], op=Alu.add)
        else:
            nc.vector._custom_dve(
                CLUSTER_UNTAG_ANT,
                out=XO[:, :, :],
                in0=XO[:, :, :],
                s0=SQ_THR,
                s1=UNTAG,
            )
        dst = out_ap[r0 : r0 + lines_per_tile, :].rearrange("(p g) c -> p g c", p=128)
        nc.sync.dma_start(out=dst, in_=XO[:, :, :])
        r0 += lines_per_tile


def _split_multi_waits(nc: bass.Bass, max_waits: int = 1) -> None:
    """walrus CoreV3 codegen rejects instructions with more than one or two
    sync-wait conditions ("Too many sync wait commands"). Split extra waits
    onto single-wait NOPs inserted just before the instruction (same engine,
    same block) — semantically identical for monotonic semaphores."""

    def walk(blocks):
        for bb in blocks:
            yield bb
            sub = getattr(bb, "blocks", None)
            if sub:
                yield from walk(sub)

    for fn in nc.m.functions:
        for bb in walk(fn.blocks):
            out = []
            changed = False
            for inst in bb.instructions:
                si = inst.sync_info
                if si is not None and si.on_wait and len(si.on_wait) > max_waits:
                    waits = list(si.on_wait)
                    head, tail = waits[:-max_waits], waits[-max_waits:]
                    for k, w in enumerate(head):
                        out.append(
                            mybir.InstNoOp(
                                name=f"{inst.name}-w{k}",
                                engine=inst.engine,
                                bass_nofuse=True,
                                sync_info=mybir.SyncInfo(on_wait=[w], on_update=[]),
                            )
                        )
                    inst.sync_info = mybir.SyncInfo(
                        on_wait=tail, on_update=list(si.on_update)
                    )
                    changed = True
                out.append(inst)
            if changed:
                bb.instructions = out


def build_program(
    lines_per_core: int = LINES_PER_CORE, G: int = 98, bufs: int = 2,
    untag_on_gpsimd: bool = False, split_first: bool = False,
) -> bass.Bass:
    assert lines_per_core % (128 * G) == 0
    n_tiles = lines_per_core // (128 * G)
    nc = bass.Bass("TRN2", target_bir_lowering=False, debug=False)
    xin = nc.declare_dram_parameter("xin", [lines_per_core, CL], F32, isOutput=False)
    yout = nc.declare_dram_parameter("yout", [lines_per_core, CL], F32, isOutput=True)
    with tile.TileContext(nc) as tc:
        _cluster_kernel(tc, yout, xin, G, n_tiles, bufs=bufs,
                        untag_on_gpsimd=untag_on_gpsimd, split_first=split_first)
    # Raw Bass skips the pass that populates .instr bytes for InstISA
    # subclasses (incl. InstCustomDveAnt); without it walrus codegen fails
    # with "ISA wrong length" (see library_overlay.lower_extended_insts).
    mybir.codegen_inst_isa_subclasses(nc)
    _split_multi_waits(nc)
    return nc


_PROGRAM_CACHE: dict = {}


def _get_program(
    lines_per_core: int, G: int, bufs: int = 2, untag_on_gpsimd: bool = False,
    split_first: bool = False,
) -> bass.Bass:
    key = (lines_per_core, G, bufs, untag_on_gpsimd, split_first)
    if key not in _PROGRAM_CACHE:
        _PROGRAM_CACHE[key] = build_program(
            lines_per_core, G, bufs, untag_on_gpsimd, split_first
        )
    return _PROGRAM_CACHE[key]


def run_sharded(flat_lines: np.ndarray, G: int = 98, trace: bool = False, bufs: int = 4,
                untag_on_gpsimd: bool = False, split_first: bool = False):
    """flat_lines: (n_lines, 64) fp32 with n_lines divisible by N_CORES*128*G.
    Returns (out_lines, BassKernelResults)."""
    n_lines = flat_lines.shape[0]
    lines_per_core = n_lines // N_CORES
    nc = _get_program(lines_per_core, G, bufs, untag_on_gpsimd, split_first)
    in_maps = [
        {"xin": np.ascontiguousarray(flat_lines[c * lines_per_core : (c + 1) * lines_per_core])}
        for c in range(N_CORES)
    ]
    res = run_bass_kernel_spmd(nc, in_maps, list(range(N_CORES)), trace=trace)
    out = np.concatenate([res.results[c]["yout"] for c in range(N_CORES)], axis=0)
    return out, res


def kernel(x: np.ndarray) -> np.ndarray:
    x = np.ascontiguousarray(x, dtype=np.float32)
    flat = x.reshape(-1, CL)
    out, _ = run_sharded(flat, G=98, bufs=4, trace=False)
    return out.reshape(FULL_SHAPE).astype(np.float32)
